# revision 15
# baseline (speedup 1.0000x reference)
"""Trainium2 Bass kernel for nn_Attention_41102837023186 (sparse GQA attention).

Head-tensor-parallel over 8 NeuronCores: core c owns q heads [3c, 3c+3) and
kv head c. Per core: rms-norm folded into weights/scales, QKV projections
(fp32r), RoPE+q/k-rms on DVE/ACT, block-sparse attention with the ragged-range
mask, chunked AllGather of the attention output, then the Wproj column block.

kernel(**inputs) takes the FULL unsharded inputs and returns the FULL output.
"""

import numpy as np

FULL_CFG = dict(S=3072, H=3072, HQ=24, HKV=8, D=128)
NCORES = 8
SC = 512  # token chunk (free-dim tile)
EPS = 1e-6
NEG = -1e30

_uid = [0]


# ---------------------------------------------------------------------------
# BIR post-fix: this walrus build accepts only ONE sem wait per instruction;
# Tile emits more (tail drain, DMA fan-ins). Split overflow waits onto
# preceding NoOp instructions on the same engine.
# ---------------------------------------------------------------------------
def _fix_bir_json_bytes(raw: bytes) -> bytes:
    import json as _json

    m = _json.loads(raw)
    changed = False
    for f in m.get("functions", []):
        for blk in f.get("blocks", []):
            out = []
            for inst in blk["instructions"]:
                si = inst.get("sync_info") or {}
                waits = si.get("on_wait") or []
                if len(waits) > 1:
                    changed = True
                    for w in waits[:-1]:
                        _uid[0] += 1
                        out.append(
                            {
                                "name": f"I-waitsplit-{_uid[0]}",
                                "opcode": "NoOp",
                                "engine": inst["engine"],
                                "ins": [],
                                "outs": [],
                                "debug": inst.get("debug", 0),
                                "sync_info": {"on_update": [], "on_wait": [w]},
                            }
                        )
                    si = dict(si)
                    si["on_wait"] = waits[-1:]
                    inst = dict(inst)
                    inst["sync_info"] = si
                out.append(inst)
            blk["instructions"] = out
    if not changed:
        return raw
    return _json.dumps(m).encode()


def _patch_bass(nc):
    import types

    orig = nc.to_json_bytes

    def patched(self):
        return _fix_bir_json_bytes(orig())

    nc.to_json_bytes = types.MethodType(patched, nc)
    return nc


# ---------------------------------------------------------------------------
# Host-side prep: fold norm weights, transpose layouts, range -> tile map
# ---------------------------------------------------------------------------
def _host_prep(x, cos, sin, pre_norm_w, q_norm_w, k_norm_w, Wq, Wk, Wv, Wproj,
               q_ranges, k_ranges, cfg):
    S, H, HQ, HKV, D = cfg["S"], cfg["H"], cfg["HQ"], cfg["HKV"], cfg["D"]
    HALF = D // 2
    NHQ = HQ // NCORES
    HD = HQ * D
    f32 = np.float32

    x = np.asarray(x, f32)
    cos2 = np.asarray(cos, f32).reshape(S, HALF)
    sin2 = np.asarray(sin, f32).reshape(S, HALF)
    w1 = (np.asarray(pre_norm_w, f32) + 1.0)
    qw1 = (np.asarray(q_norm_w, f32) + 1.0)
    kw1 = (np.asarray(k_norm_w, f32) + 1.0)
    Wq = np.asarray(Wq, f32) * w1[None, :]
    Wk = np.asarray(Wk, f32) * w1[None, :]
    Wv = np.asarray(Wv, f32) * w1[None, :]
    Wproj = np.asarray(Wproj, f32)
    qr = np.asarray(q_ranges).astype(np.int64)
    kr = np.asarray(k_ranges).astype(np.int64)

    xT = np.ascontiguousarray(x.T)  # [H, S]

    # rope packs [D, S]: rows 0:HALF scale for x_lo terms, HALF:D for x_hi
    def pack(tab, wvec):
        return np.ascontiguousarray(
            np.concatenate([tab.T * wvec[:HALF, None], tab.T * wvec[HALF:, None]],
                           axis=0)).astype(f32)

    cospack_q, sinpack_q = pack(cos2, qw1), pack(sin2, qw1)
    cospack_k, sinpack_k = pack(cos2, kw1), pack(sin2, kw1)

    # ragged-range tile map in scores^T orientation: allowed[k, q]
    allowed = np.zeros((S, S), dtype=bool)
    covered = np.zeros((S,), dtype=bool)
    for r in range(qr.shape[0]):
        q0, q1 = int(qr[r, 0]), int(qr[r, 1])
        k0, k1 = int(kr[r, 0]), int(kr[r, 1])
        q0, q1 = max(q0, 0), min(q1, S)
        k0, k1 = max(k0, 0), min(k1, S)
        if q1 > q0:
            covered[q0:q1] = True
            if k1 > k0:
                allowed[k0:k1, q0:q1] = True

    n_kt = S // D
    n_sc = S // SC
    masks = []
    chunk_plan = []  # per sc: list of (kt, mask_id_or_None)
    uncov_needed = []
    for sc in range(n_sc):
        plan = []
        qs = slice(sc * SC, (sc + 1) * SC)
        for kt in range(n_kt):
            sub = allowed[kt * D:(kt + 1) * D, qs]
            if sub.all():
                plan.append((kt, None))
            elif sub.any():
                masks.append(np.where(sub, np.float32(0), np.float32(NEG)))
                plan.append((kt, len(masks) - 1))
        chunk_plan.append(plan)
        cov_chunk = covered[qs]
        # den += 1 where this chunk's q has no allowed keys (avoid 0*inf)
        has_keys = allowed[:, qs].any(axis=0)
        uncov_needed.append(None if has_keys.all()
                            else (~has_keys).astype(f32)[None, :])

    masks_arr = (np.ascontiguousarray(np.stack(masks)) if masks
                 else np.zeros((1, D, SC), f32))

    cov_arr = covered.astype(f32)[None, :]  # [1, S], for output zeroing

    per_core = []
    for c in range(NCORES):
        wkvq = np.ascontiguousarray(
            np.concatenate(
                [Wk[c * D:(c + 1) * D].T, Wv[c * D:(c + 1) * D].T,
                 Wq[c * NHQ * D:(c + 1) * NHQ * D].T], axis=1)).astype(f32)
        outc = H // NCORES
        wpt = np.ascontiguousarray(
            Wproj[c * outc:(c + 1) * outc].T).astype(f32)  # [HD, H//NCORES]
        per_core.append(dict(xT=xT, wkvq=wkvq, wpt=wpt,
                             cospack_q=cospack_q, sinpack_q=sinpack_q,
                             cospack_k=cospack_k, sinpack_k=sinpack_k,
                             masks=masks_arr))
    spec = dict(chunk_plan=chunk_plan, uncov=uncov_needed, covered=cov_arr,
                all_covered=bool(covered.all()))
    return per_core, spec


# ---------------------------------------------------------------------------
# Device program (identical on all cores; SPMD over inputs)
# ---------------------------------------------------------------------------
def _build_program(cfg, spec, n_masks):
    import concourse.bass as bass
    import concourse.tile as tile
    from concourse import mybir

    f32 = mybir.dt.float32
    f32r = mybir.dt.float32r
    AF = mybir.ActivationFunctionType

    S, H, HQ, HKV, D = cfg["S"], cfg["H"], cfg["HQ"], cfg["HKV"], cfg["D"]
    HALF = D // 2
    NHQ = HQ // NCORES
    HD = HQ * D
    n_ht = H // D
    n_kt = S // D
    n_sc = S // SC
    n_st = SC // D  # 128-token subtiles per chunk
    OUTC = H // NCORES  # output columns per core
    QKSCALE = float(1.0 / np.sqrt(D))
    chunk_plan = spec["chunk_plan"]
    uncov = spec["uncov"]

    nc = bass.Bass(num_devices=NCORES)

    # register EPS as a const AP so activation(bias=EPS) can resolve it
    _epst = nc.alloc_sbuf_tensor("const-float32-eps", [128, 1], f32)
    nc.gpsimd.memset(_epst.ap(), EPS)
    nc.const_aps.aps[(f32, EPS)] = _epst.ap()
    nc.all_engine_barrier()

    xT_d = nc.dram_tensor("xT", [H, S], f32, kind="ExternalInput")
    wkvq_d = nc.dram_tensor("wkvq", [H, (2 + NHQ) * D], f32, kind="ExternalInput")
    wpt_d = nc.dram_tensor("wpt", [HD, OUTC], f32, kind="ExternalInput")
    cq_d = nc.dram_tensor("cospack_q", [D, S], f32, kind="ExternalInput")
    sq_d = nc.dram_tensor("sinpack_q", [D, S], f32, kind="ExternalInput")
    ck_d = nc.dram_tensor("cospack_k", [D, S], f32, kind="ExternalInput")
    sk_d = nc.dram_tensor("sinpack_k", [D, S], f32, kind="ExternalInput")
    masks_d = nc.dram_tensor("masks", [n_masks, D, SC], f32, kind="ExternalInput")
    out_d = nc.dram_tensor("out", [S, OUTC], f32, kind="ExternalOutput")

    r_dram = nc.dram_tensor("r_scratch", [1, S], f32)
    ag_in = [nc.dram_tensor(f"ag_in_{j}", [NHQ * D, SC], f32) for j in range(n_sc)]
    ag_out = [nc.dram_tensor(f"ag_out_{j}", [HD, SC], f32, addr_space="Shared")
              for j in range(n_sc)]

    uncov_d = None
    if any(u is not None for u in uncov):
        uncov_d = nc.dram_tensor("uncov", [1, S], f32, kind="ExternalInput")

    ident_d = nc.inline_tensor(np.eye(D, dtype=np.float32), name="ident128")
    ones_d = nc.inline_tensor(np.ones((D, 1), dtype=np.float32), name="ones128")
    onesr_d = nc.inline_tensor(np.ones((1, D), dtype=np.float32), name="ones1x128")

    from contextlib import ExitStack
    with tile.TileContext(nc) as tc, ExitStack() as ctx:
        pool = lambda *a, **k: ctx.enter_context(tc.tile_pool(*a, **k))
        const_p = pool(name="const", bufs=1)
        w_p = pool(name="wkvq", bufs=n_ht)
        wpt_p = pool(name="wpt", bufs=HD // D)
        big_p = pool(name="big", bufs=1)
        x_p = pool(name="x", bufs=3)
        sq_p = pool(name="sq", bufs=3)
        trig_p = pool(name="trig", bufs=2)
        rope_p = pool(name="rope", bufs=4)
        tmp_p = pool(name="tmp", bufs=2)
        qh_p = pool(name="qh", bufs=3)
        pexp_p = pool(name="pexp", bufs=2)
        row_p = pool(name="row", bufs=3)
        rb_p = pool(name="rb", bufs=2)
        at_p = pool(name="at", bufs=2)
        lt_p = pool(name="lt", bufs=2)
        os_p = pool(name="os", bufs=2)
        any_masks = any(mid is not None for plan in chunk_plan for _, mid in plan)
        mask_p = pool(name="mask", bufs=2) if any_masks else None
        psN = pool(name="psN", bufs=6, space="PSUM")
        psW = pool(name="psW", bufs=1, space="PSUM")

        ident = const_p.tile([D, D], f32r)
        nc.sync.dma_start(ident[:], ident_d.ap().bitcast(f32r))
        ones = const_p.tile([D, 1], f32r)
        nc.sync.dma_start(ones[:], ones_d.ap().bitcast(f32r))
        onesr = const_p.tile([1, D], f32r)
        nc.sync.dma_start(onesr[:], onesr_d.ap().bitcast(f32r))

        wkvq_sb = []
        for t in range(n_ht):
            w = w_p.tile([D, (2 + NHQ) * D], f32r, tag="w")
            nc.sync.dma_start(w[:], wkvq_d[t * D:(t + 1) * D, :].bitcast(f32r))
            wkvq_sb.append(w)

        khatT = big_p.tile([D, S], f32r, tag="khat")   # [d, token]
        v_sb = big_p.tile([D, S], f32r, tag="v")       # [token(kt-major), d]

        # per-partition r: r_pp[p, t] = r_row[t*128 + p]
        r_pp = big_p.tile([D, n_kt], f32, tag="rpp")

        uncov_sb = None
        if uncov_d is not None:
            uncov_sb = big_p.tile([1, S], f32, tag="uncov")
            nc.sync.dma_start(uncov_sb[:], uncov_d[:, :])

        def rope_block(psrc, cos_t, sin_t, dst_ap, scale_sb):
            """dst = rope(psrc) * scale; psrc is PSUM [D, SC], cos/sin packs
            [D, SC] in SBUF, scale_sb [D(bcast rows), SC] f32 SBUF."""
            t1 = rope_p.tile([HALF, SC], f32, tag="rp")
            t2 = rope_p.tile([HALF, SC], f32, tag="rp")
            t3 = rope_p.tile([HALF, SC], f32, tag="rp")
            t4 = rope_p.tile([HALF, SC], f32, tag="rp")
            nc.vector.tensor_mul(t1[:], psrc[0:HALF, :], cos_t[0:HALF, :])
            nc.vector.tensor_mul(t2[:], psrc[HALF:D, :], sin_t[HALF:D, :])
            nc.vector.tensor_mul(t3[:], psrc[HALF:D, :], cos_t[HALF:D, :])
            nc.vector.tensor_mul(t4[:], psrc[0:HALF, :], sin_t[0:HALF, :])
            tmp = tmp_p.tile([D, SC], f32, tag="ropetmp")
            nc.vector.tensor_sub(tmp[0:HALF, :], t1[:], t2[:])
            nc.vector.tensor_add(tmp[HALF:D, :], t3[:], t4[:])
            nc.vector.tensor_mul(dst_ap, tmp[:], scale_sb)

        def rms_scale(p_raw, name_tag):
            """per-token rsqrt(mean_d(p_raw^2)+eps) broadcast to [D, SC] f32."""
            sq = sq_p.tile([D, SC], f32r, tag="sq", bufs=2)
            nc.scalar.activation(sq[:], p_raw[:], AF.Square)
            pss = psN.tile([1, SC], f32, tag="b")
            nc.tensor.matmul(pss[:], ones[:], sq[:], start=True, stop=True)
            tvar = row_p.tile([1, SC], f32, tag="row")
            nc.scalar.activation(tvar[:], pss[:], AF.Ln, scale=1.0 / D, bias=EPS)
            rq = row_p.tile([1, SC], f32r, tag="rowr")
            nc.scalar.activation(rq[:], tvar[:], AF.Exp, scale=-0.5)
            prb = psN.tile([D, SC], f32, tag="b")
            nc.tensor.matmul(prb[:], onesr[:], rq[:], start=True, stop=True)
            rb = rb_p.tile([D, SC], f32, tag="rb")
            nc.scalar.copy(rb[:], prb[:])
            return rb

        # ---------------- pass 1: K/V projections + x sumsq ----------------
        for sc in range(n_sc):
            ssl = slice(sc * SC, (sc + 1) * SC)
            pk = psN.tile([D, SC], f32, tag="b")
            pv = psN.tile([D, SC], f32, tag="b")
            pss = psN.tile([1, SC], f32, tag="b")
            for ht in range(n_ht):
                xt = x_p.tile([D, SC], f32r, tag="x")
                nc.sync.dma_start(xt[:], xT_d[ht * D:(ht + 1) * D, ssl].bitcast(f32r))
                st, sp = ht == 0, ht == n_ht - 1
                nc.tensor.matmul(pk[:], wkvq_sb[ht][:, 0:D], xt[:], start=st, stop=sp)
                nc.tensor.matmul(pv[:], wkvq_sb[ht][:, D:2 * D], xt[:], start=st, stop=sp)
                sqx = sq_p.tile([D, SC], f32r, tag="sqx")
                nc.vector.tensor_mul(sqx[:], xt[:].bitcast(f32), xt[:].bitcast(f32))
                nc.tensor.matmul(pss[:], ones[:], sqx[:], start=st, stop=sp)
            # r chunk: exp(±0.5 * ln(ssq/H + eps))
            tvar = row_p.tile([1, SC], f32, tag="row")
            nc.scalar.activation(tvar[:], pss[:], AF.Ln, scale=1.0 / H, bias=EPS)
            r_chunk = row_p.tile([1, SC], f32, tag="row")
            nc.scalar.activation(r_chunk[:], tvar[:], AF.Exp, scale=-0.5)
            # reshape r chunk to per-partition layout for the v-copy scale
            # (bounce via DRAM: SBUF APs cannot re-partition in one DMA)
            nc.sync.dma_start(r_dram[0:1, ssl], r_chunk[:])
            nc.sync.dma_start(
                r_pp[:, sc * n_st:(sc + 1) * n_st],
                r_dram[0:1, ssl].rearrange("o (j p) -> (o p) j", p=D))
            # k: rms + rope
            ckt = trig_p.tile([D, SC], f32, tag="ck")
            skt = trig_p.tile([D, SC], f32, tag="sk")
            nc.sync.dma_start(ckt[:], ck_d[:, ssl])
            nc.sync.dma_start(skt[:], sk_d[:, ssl])
            rb = rms_scale(pk, "k")
            rope_block(pk, ckt, skt, khatT[:, ssl], rb[:])
            # v: copy (unnormalized) then transpose to [token, d]
            vt = tmp_p.tile([D, SC], f32r, tag="vt")
            nc.scalar.copy(vt[:], pv[:])
            for j in range(n_st):
                ptr = psN.tile([D, D], f32r, tag="b")
                nc.tensor.transpose(ptr[:], vt[:, j * D:(j + 1) * D], ident[:])
                kt = sc * n_st + j
                # scale by r[token] during PSUM->SBUF copy (token = partition)
                nc.scalar.activation(v_sb[:, kt * D:(kt + 1) * D],
                                     ptr[:].bitcast(f32), AF.Copy,
                                     scale=r_pp[:, kt:kt + 1])

        # ---------------- pass 2: Q proj + attention + AG + proj ----------
        wpt_sb = []
        for t in range(HD // D):
            w = wpt_p.tile([D, OUTC], f32r, tag="wp")
            nc.sync.dma_start(w[:], wpt_d[t * D:(t + 1) * D, :].bitcast(f32r))
            wpt_sb.append(w)

        for sc in range(n_sc):
            ssl = slice(sc * SC, (sc + 1) * SC)
            pq = [psN.tile([D, SC], f32, tag="b", name=f"pq{_h}") for _h in range(NHQ)]
            for ht in range(n_ht):
                xt = x_p.tile([D, SC], f32r, tag="x")
                nc.sync.dma_start(xt[:], xT_d[ht * D:(ht + 1) * D, ssl].bitcast(f32r))
                st, sp = ht == 0, ht == n_ht - 1
                for h in range(NHQ):
                    nc.tensor.matmul(pq[h][:], wkvq_sb[ht][:, (2 + h) * D:(3 + h) * D],
                                     xt[:], start=st, stop=sp)
            cqt = trig_p.tile([D, SC], f32, tag="ck")
            sqt = trig_p.tile([D, SC], f32, tag="sk")
            nc.sync.dma_start(cqt[:], cq_d[:, ssl])
            nc.sync.dma_start(sqt[:], sq_d[:, ssl])
            qhat = []
            for h in range(NHQ):
                rb = rms_scale(pq[h], "q")
                qh = qh_p.tile([D, SC], f32r, tag="qh")
                rope_block(pq[h], cqt, sqt, qh[:], rb[:])
                qhat.append(qh)

            plan = chunk_plan[sc]
            for h in range(NHQ):
                pattn = psN.tile([D, SC], f32, tag="b")
                pden = psN.tile([1, SC], f32, tag="b")
                first = True
                i = 0
                while i < len(plan):
                    pair = plan[i:i + 2]
                    ps = psW.tile([D, 2 * SC], f32, tag="s")
                    for pi, (kt, mid) in enumerate(pair):
                        col = slice(pi * SC, (pi + 1) * SC)
                        nc.tensor.matmul(ps[:, col], khatT[:, kt * D:(kt + 1) * D],
                                         qhat[h][:], start=True, stop=True)
                        if mid is not None:
                            mt = mask_p.tile([D, SC], f32, tag="m")
                            nc.sync.dma_start(mt[:], masks_d[mid, :, :])
                            nc.vector.tensor_add(ps[:, col], ps[:, col], mt[:])
                    pe = pexp_p.tile([D, 2 * SC], f32r, tag="pe")
                    ncols = len(pair) * SC
                    nc.scalar.activation(pe[:, 0:ncols], ps[:, 0:ncols], AF.Exp,
                                         scale=QKSCALE)
                    for pi, (kt, mid) in enumerate(pair):
                        col = slice(pi * SC, (pi + 1) * SC)
                        last = (i + pi) == len(plan) - 1
                        nc.tensor.matmul(pattn[:], v_sb[:, kt * D:(kt + 1) * D],
                                         pe[:, col], start=first, stop=last)
                        nc.tensor.matmul(pden[:], ones[:], pe[:, col],
                                         start=first, stop=last)
                        first = False
                    i += 2

                at = at_p.tile([D, SC], f32, tag="at")
                if not plan:
                    nc.vector.memset(at[:], 0.0)
                else:
                    if uncov[sc] is not None:
                        nc.vector.tensor_add(pden[:], pden[:], uncov_sb[0:1, ssl])
                    rec = row_p.tile([1, SC], f32r, tag="rowr")
                    with nc.allow_low_precision(reason="f32r broadcast rhs"):
                        nc.vector.reciprocal(rec[:], pden[:])
                    prb = psN.tile([D, SC], f32, tag="b")
                    nc.tensor.matmul(prb[:], onesr[:], rec[:], start=True, stop=True)
                    rb2 = rb_p.tile([D, SC], f32, tag="rb")
                    nc.scalar.copy(rb2[:], prb[:])
                    nc.vector.tensor_mul(at[:], pattn[:], rb2[:])
                nc.sync.dma_start(ag_in[sc][h * D:(h + 1) * D, :], at[:])

            nc.gpsimd.collective_compute(
                "AllGather", mybir.AluOpType.bypass,
                replica_groups=[list(range(NCORES))],
                ins=[ag_in[sc].ap()], outs=[ag_out[sc].ap()],
            )

            # output projection for this token chunk
            po = [psN.tile([D, OUTC], f32, tag="b", name=f"po{_j}") for _j in range(n_st)]
            for t in range(HD // D):
                lt = lt_p.tile([D, SC], f32r, tag="lt")
                nc.sync.dma_start(lt[:], ag_out[sc][t * D:(t + 1) * D, :].bitcast(f32r))
                for j in range(n_st):
                    nc.tensor.matmul(po[j][:], lt[:, j * D:(j + 1) * D], wpt_sb[t][:],
                                     start=(t == 0), stop=(t == HD // D - 1))
            for j in range(n_st):
                ob = os_p.tile([D, OUTC], f32, tag="os")
                nc.scalar.copy(ob[:], po[j][:])
                nc.sync.dma_start(out_d[sc * SC + j * D: sc * SC + (j + 1) * D, :],
                                  ob[:])

    return nc


def build_and_run(x, cos, sin, pre_norm_w, q_norm_w, k_norm_w, Wq, Wk, Wv,
                  Wproj, q_ranges, k_ranges, cfg=None, trace=False,
                  trace_kwargs=None):
    from concourse.bass_utils import run_bass_kernel_spmd

    cfg = cfg or FULL_CFG
    per_core, spec = _host_prep(x, cos, sin, pre_norm_w, q_norm_w, k_norm_w,
                                Wq, Wk, Wv, Wproj, q_ranges, k_ranges, cfg)
    n_masks = per_core[0]["masks"].shape[0]
    nc = _build_program(cfg, spec, n_masks)
    _patch_bass(nc)

    in_maps = []
    for c in range(NCORES):
        m = dict(per_core[c])
        if any(u is not None for u in spec["uncov"]):
            S = cfg["S"]
            ua = np.zeros((1, S), np.float32)
            for sc, u in enumerate(spec["uncov"]):
                if u is not None:
                    ua[0, sc * SC:(sc + 1) * SC] = u
            m["uncov"] = ua
        in_maps.append(m)

    kw = {}
    if trace:
        kw = dict(trace=True, trace_kwargs=trace_kwargs or {})
    res = run_bass_kernel_spmd(nc, in_maps, core_ids=list(range(NCORES)), **kw)
    out = np.concatenate([res.results[c]["out"] for c in range(NCORES)], axis=1)
    if not spec["all_covered"]:
        out = out * spec["covered"].T  # zero uncovered rows
    return out, res


def kernel(**inputs):
    out, _ = build_and_run(**inputs)
    return out


# revision 16
# speedup vs baseline: 1.2155x; 1.2155x over previous
"""Trainium2 Bass kernel for nn_Attention_41102837023186 (sparse GQA attention).

Head-tensor-parallel over 8 NeuronCores: core c owns q heads [3c, 3c+3) and
kv head c. Per core: rms-norm folded into weights/scales, QKV projections
(fp32r), RoPE+q/k-rms on DVE/ACT, block-sparse attention with the ragged-range
mask, chunked AllGather of the attention output, then the Wproj column block.

kernel(**inputs) takes the FULL unsharded inputs and returns the FULL output.
"""

import numpy as np

FULL_CFG = dict(S=3072, H=3072, HQ=24, HKV=8, D=128)
NCORES = 8
SC = 512  # token chunk (free-dim tile)
EPS = 1e-6
NEG = -1e30

_uid = [0]


# ---------------------------------------------------------------------------
# BIR post-fix: this walrus build accepts only ONE sem wait per instruction;
# Tile emits more (tail drain, DMA fan-ins). Split overflow waits onto
# preceding NoOp instructions on the same engine.
# ---------------------------------------------------------------------------
def _fix_bir_json_bytes(raw: bytes) -> bytes:
    import json as _json

    m = _json.loads(raw)
    changed = False
    for f in m.get("functions", []):
        for blk in f.get("blocks", []):
            out = []
            for inst in blk["instructions"]:
                si = inst.get("sync_info") or {}
                waits = si.get("on_wait") or []
                if len(waits) > 1:
                    changed = True
                    for w in waits[:-1]:
                        _uid[0] += 1
                        out.append(
                            {
                                "name": f"I-waitsplit-{_uid[0]}",
                                "opcode": "NoOp",
                                "engine": inst["engine"],
                                "ins": [],
                                "outs": [],
                                "debug": inst.get("debug", 0),
                                "sync_info": {"on_update": [], "on_wait": [w]},
                            }
                        )
                    si = dict(si)
                    si["on_wait"] = waits[-1:]
                    inst = dict(inst)
                    inst["sync_info"] = si
                out.append(inst)
            blk["instructions"] = out
    if not changed:
        return raw
    return _json.dumps(m).encode()


def _patch_bass(nc):
    import types

    orig = nc.to_json_bytes

    def patched(self):
        return _fix_bir_json_bytes(orig())

    nc.to_json_bytes = types.MethodType(patched, nc)
    return nc


# ---------------------------------------------------------------------------
# Host-side prep: fold norm weights, transpose layouts, range -> tile map
# ---------------------------------------------------------------------------
def _host_prep(x, cos, sin, pre_norm_w, q_norm_w, k_norm_w, Wq, Wk, Wv, Wproj,
               q_ranges, k_ranges, cfg):
    S, H, HQ, HKV, D = cfg["S"], cfg["H"], cfg["HQ"], cfg["HKV"], cfg["D"]
    HALF = D // 2
    NHQ = HQ // NCORES
    HD = HQ * D
    f32 = np.float32

    x = np.asarray(x, f32)
    cos2 = np.asarray(cos, f32).reshape(S, HALF)
    sin2 = np.asarray(sin, f32).reshape(S, HALF)
    w1 = (np.asarray(pre_norm_w, f32) + 1.0)
    qw1 = (np.asarray(q_norm_w, f32) + 1.0)
    kw1 = (np.asarray(k_norm_w, f32) + 1.0)
    Wq = np.asarray(Wq, f32) * w1[None, :]
    Wk = np.asarray(Wk, f32) * w1[None, :]
    Wv = np.asarray(Wv, f32) * w1[None, :]
    Wproj = np.asarray(Wproj, f32)
    qr = np.asarray(q_ranges).astype(np.int64)
    kr = np.asarray(k_ranges).astype(np.int64)

    xT = np.ascontiguousarray(x.T)  # [H, S]

    # rope packs [D, S]: rows 0:HALF scale for x_lo terms, HALF:D for x_hi
    def pack(tab, wvec):
        return np.ascontiguousarray(
            np.concatenate([tab.T * wvec[:HALF, None], tab.T * wvec[HALF:, None]],
                           axis=0)).astype(f32)

    cospack_q, sinpack_q = pack(cos2, qw1), pack(sin2, qw1)
    cospack_k, sinpack_k = pack(cos2, kw1), pack(sin2, kw1)

    # ragged-range tile map in scores^T orientation: allowed[k, q]
    allowed = np.zeros((S, S), dtype=bool)
    covered = np.zeros((S,), dtype=bool)
    for r in range(qr.shape[0]):
        q0, q1 = int(qr[r, 0]), int(qr[r, 1])
        k0, k1 = int(kr[r, 0]), int(kr[r, 1])
        q0, q1 = max(q0, 0), min(q1, S)
        k0, k1 = max(k0, 0), min(k1, S)
        if q1 > q0:
            covered[q0:q1] = True
            if k1 > k0:
                allowed[k0:k1, q0:q1] = True

    n_kt = S // D
    n_sc = S // SC
    masks = []
    chunk_plan = []  # per sc: list of (kt, mask_id_or_None)
    uncov_needed = []
    for sc in range(n_sc):
        plan = []
        qs = slice(sc * SC, (sc + 1) * SC)
        for kt in range(n_kt):
            sub = allowed[kt * D:(kt + 1) * D, qs]
            if sub.all():
                plan.append((kt, None))
            elif sub.any():
                masks.append(np.where(sub, np.float32(0), np.float32(NEG)))
                plan.append((kt, len(masks) - 1))
        chunk_plan.append(plan)
        cov_chunk = covered[qs]
        # den += 1 where this chunk's q has no allowed keys (avoid 0*inf)
        has_keys = allowed[:, qs].any(axis=0)
        uncov_needed.append(None if has_keys.all()
                            else (~has_keys).astype(f32)[None, :])

    masks_arr = (np.ascontiguousarray(np.stack(masks)) if masks
                 else np.zeros((1, D, SC), f32))

    cov_arr = covered.astype(f32)[None, :]  # [1, S], for output zeroing

    per_core = []
    for c in range(NCORES):
        wkvq = np.ascontiguousarray(
            np.concatenate(
                [Wk[c * D:(c + 1) * D].T, Wv[c * D:(c + 1) * D].T,
                 Wq[c * NHQ * D:(c + 1) * NHQ * D].T], axis=1)).astype(f32)
        outc = H // NCORES
        wpt = np.ascontiguousarray(
            Wproj[c * outc:(c + 1) * outc].T).astype(f32)  # [HD, H//NCORES]
        per_core.append(dict(xT=xT, wkvq=wkvq, wpt=wpt,
                             cospack_q=cospack_q, sinpack_q=sinpack_q,
                             cospack_k=cospack_k, sinpack_k=sinpack_k,
                             masks=masks_arr))
    spec = dict(chunk_plan=chunk_plan, uncov=uncov_needed, covered=cov_arr,
                all_covered=bool(covered.all()))
    return per_core, spec


# ---------------------------------------------------------------------------
# Device program (identical on all cores; SPMD over inputs)
# ---------------------------------------------------------------------------
def _build_program(cfg, spec, n_masks):
    import concourse.bass as bass
    import concourse.tile as tile
    from concourse import mybir

    f32 = mybir.dt.float32
    f32r = mybir.dt.float32r
    AF = mybir.ActivationFunctionType

    S, H, HQ, HKV, D = cfg["S"], cfg["H"], cfg["HQ"], cfg["HKV"], cfg["D"]
    HALF = D // 2
    NHQ = HQ // NCORES
    HD = HQ * D
    n_ht = H // D
    n_kt = S // D
    n_sc = S // SC
    n_st = SC // D  # 128-token subtiles per chunk
    OUTC = H // NCORES  # output columns per core
    QKSCALE = float(1.0 / np.sqrt(D))
    chunk_plan = spec["chunk_plan"]
    uncov = spec["uncov"]

    nc = bass.Bass(num_devices=NCORES)

    # register EPS as a const AP so activation(bias=EPS) can resolve it
    _epst = nc.alloc_sbuf_tensor("const-float32-eps", [128, 1], f32)
    nc.gpsimd.memset(_epst.ap(), EPS)
    nc.const_aps.aps[(f32, EPS)] = _epst.ap()
    nc.all_engine_barrier()

    xT_d = nc.dram_tensor("xT", [H, S], f32, kind="ExternalInput")
    wkvq_d = nc.dram_tensor("wkvq", [H, (2 + NHQ) * D], f32, kind="ExternalInput")
    wpt_d = nc.dram_tensor("wpt", [HD, OUTC], f32, kind="ExternalInput")
    cq_d = nc.dram_tensor("cospack_q", [D, S], f32, kind="ExternalInput")
    sq_d = nc.dram_tensor("sinpack_q", [D, S], f32, kind="ExternalInput")
    ck_d = nc.dram_tensor("cospack_k", [D, S], f32, kind="ExternalInput")
    sk_d = nc.dram_tensor("sinpack_k", [D, S], f32, kind="ExternalInput")
    masks_d = nc.dram_tensor("masks", [n_masks, D, SC], f32, kind="ExternalInput")
    out_d = nc.dram_tensor("out", [OUTC, S], f32, kind="ExternalOutput")

    r_dram = nc.dram_tensor("r_scratch", [1, S], f32)
    ag_in = [nc.dram_tensor(f"ag_in_{j}", [NHQ * D, SC], f32) for j in range(n_sc)]
    ag_out = [nc.dram_tensor(f"ag_out_{j}", [HD, SC], f32, addr_space="Shared")
              for j in range(n_sc)]

    uncov_d = None
    if any(u is not None for u in uncov):
        uncov_d = nc.dram_tensor("uncov", [1, S], f32, kind="ExternalInput")

    ident_d = nc.inline_tensor(np.eye(D, dtype=np.float32), name="ident128")
    ones_d = nc.inline_tensor(np.ones((D, 1), dtype=np.float32), name="ones128")
    onesr_d = nc.inline_tensor(np.ones((1, D), dtype=np.float32), name="ones1x128")

    from contextlib import ExitStack
    with tile.TileContext(nc) as tc, ExitStack() as ctx:
        pool = lambda *a, **k: ctx.enter_context(tc.tile_pool(*a, **k))
        const_p = pool(name="const", bufs=1)
        w_p = pool(name="wkvq", bufs=n_ht)
        wpt_p = pool(name="wpt", bufs=HD // D)
        big_p = pool(name="big", bufs=1)
        x_p = pool(name="x", bufs=3)
        sq_p = pool(name="sq", bufs=3)
        trig_p = pool(name="trig", bufs=2)
        rope_p = pool(name="rope", bufs=4)
        tmp_p = pool(name="tmp", bufs=2)
        qh_p = pool(name="qh", bufs=4)
        pexp_p = pool(name="pexp", bufs=3)
        row_p = pool(name="row", bufs=3)
        rb_p = pool(name="rb", bufs=2)
        at_p = pool(name="at", bufs=2)
        lt_p = pool(name="lt", bufs=2)
        os_p = pool(name="os", bufs=2)
        any_masks = any(mid is not None for plan in chunk_plan for _, mid in plan)
        mask_p = pool(name="mask", bufs=1) if any_masks else None
        psN = pool(name="psN", bufs=8, space="PSUM")

        ident = const_p.tile([D, D], f32r)
        nc.sync.dma_start(ident[:], ident_d.ap().bitcast(f32r))
        ones = const_p.tile([D, 1], f32r)
        nc.sync.dma_start(ones[:], ones_d.ap().bitcast(f32r))
        onesr = const_p.tile([1, D], f32r)
        nc.sync.dma_start(onesr[:], onesr_d.ap().bitcast(f32r))

        wkvq_sb = []
        for t in range(n_ht):
            w = w_p.tile([D, (2 + NHQ) * D], f32r, tag="w")
            nc.sync.dma_start(w[:], wkvq_d[t * D:(t + 1) * D, :].bitcast(f32r))
            wkvq_sb.append(w)

        khatT = big_p.tile([D, S], f32r, tag="khat")   # [d, token]
        v_sb = big_p.tile([D, S], f32r, tag="v")       # [token(kt-major), d]

        # per-partition r: r_pp[p, t] = r_row[t*128 + p]
        r_pp = big_p.tile([D, n_kt], f32, tag="rpp")

        uncov_sb = None
        if uncov_d is not None:
            uncov_sb = big_p.tile([1, S], f32, tag="uncov")
            nc.sync.dma_start(uncov_sb[:], uncov_d[:, :])

        def rope_block(psrc, cos_t, sin_t, dst_ap, scale_sb):
            """dst = rope(psrc) * scale; psrc is PSUM [D, SC], cos/sin packs
            [D, SC] in SBUF, scale_sb [D(bcast rows), SC] f32 SBUF."""
            t1 = rope_p.tile([HALF, SC], f32, tag="rp")
            t2 = rope_p.tile([HALF, SC], f32, tag="rp")
            t3 = rope_p.tile([HALF, SC], f32, tag="rp")
            t4 = rope_p.tile([HALF, SC], f32, tag="rp")
            nc.vector.tensor_mul(t1[:], psrc[0:HALF, :], cos_t[0:HALF, :])
            nc.vector.tensor_mul(t2[:], psrc[HALF:D, :], sin_t[HALF:D, :])
            nc.vector.tensor_mul(t3[:], psrc[HALF:D, :], cos_t[HALF:D, :])
            nc.vector.tensor_mul(t4[:], psrc[0:HALF, :], sin_t[0:HALF, :])
            tmp = tmp_p.tile([D, SC], f32, tag="ropetmp")
            nc.vector.tensor_sub(tmp[0:HALF, :], t1[:], t2[:])
            nc.vector.tensor_add(tmp[HALF:D, :], t3[:], t4[:])
            nc.vector.tensor_mul(dst_ap, tmp[:], scale_sb)

        def rms_scale(p_raw, name_tag):
            """per-token rsqrt(mean_d(p_raw^2)+eps) broadcast to [D, SC] f32."""
            sq = sq_p.tile([D, SC], f32r, tag="sq", bufs=2)
            nc.scalar.activation(sq[:], p_raw[:], AF.Square)
            pss = psN.tile([1, SC], f32, tag="b")
            nc.tensor.matmul(pss[:], ones[:], sq[:], start=True, stop=True)
            tvar = row_p.tile([1, SC], f32, tag="row")
            nc.scalar.activation(tvar[:], pss[:], AF.Ln, scale=1.0 / D, bias=EPS)
            rq = row_p.tile([1, SC], f32r, tag="rowr")
            nc.scalar.activation(rq[:], tvar[:], AF.Exp, scale=-0.5)
            prb = psN.tile([D, SC], f32, tag="b")
            nc.tensor.matmul(prb[:], onesr[:], rq[:], start=True, stop=True)
            rb = rb_p.tile([D, SC], f32, tag="rb")
            nc.scalar.copy(rb[:], prb[:])
            return rb

        # ---------------- pass 1: K/V projections + x sumsq ----------------
        for sc in range(n_sc):
            ssl = slice(sc * SC, (sc + 1) * SC)
            pk = psN.tile([D, SC], f32, tag="b")
            pv = psN.tile([D, SC], f32, tag="b")
            pss = psN.tile([1, SC], f32, tag="b")
            for ht in range(n_ht):
                xt = x_p.tile([D, SC], f32r, tag="x")
                nc.sync.dma_start(xt[:], xT_d[ht * D:(ht + 1) * D, ssl].bitcast(f32r))
                st, sp = ht == 0, ht == n_ht - 1
                nc.tensor.matmul(pk[:], wkvq_sb[ht][:, 0:D], xt[:], start=st, stop=sp)
                nc.tensor.matmul(pv[:], wkvq_sb[ht][:, D:2 * D], xt[:], start=st, stop=sp)
                sqx = sq_p.tile([D, SC], f32r, tag="sqx")
                nc.vector.tensor_mul(sqx[:], xt[:].bitcast(f32), xt[:].bitcast(f32))
                nc.tensor.matmul(pss[:], ones[:], sqx[:], start=st, stop=sp)
            # r chunk: exp(±0.5 * ln(ssq/H + eps))
            tvar = row_p.tile([1, SC], f32, tag="row")
            nc.scalar.activation(tvar[:], pss[:], AF.Ln, scale=1.0 / H, bias=EPS)
            r_chunk = row_p.tile([1, SC], f32, tag="row")
            nc.scalar.activation(r_chunk[:], tvar[:], AF.Exp, scale=-0.5)
            # reshape r chunk to per-partition layout for the v-copy scale
            # (bounce via DRAM: SBUF APs cannot re-partition in one DMA)
            nc.sync.dma_start(r_dram[0:1, ssl], r_chunk[:])
            nc.sync.dma_start(
                r_pp[:, sc * n_st:(sc + 1) * n_st],
                r_dram[0:1, ssl].rearrange("o (j p) -> (o p) j", p=D))
            # k: rms + rope
            ckt = trig_p.tile([D, SC], f32, tag="ck")
            skt = trig_p.tile([D, SC], f32, tag="sk")
            nc.sync.dma_start(ckt[:], ck_d[:, ssl])
            nc.sync.dma_start(skt[:], sk_d[:, ssl])
            rb = rms_scale(pk, "k")
            rope_block(pk, ckt, skt, khatT[:, ssl], rb[:])
            # v: copy (unnormalized) then transpose to [token, d]
            vt = tmp_p.tile([D, SC], f32r, tag="vt")
            nc.scalar.copy(vt[:], pv[:])
            for j in range(n_st):
                ptr = psN.tile([D, D], f32r, tag="b")
                nc.tensor.transpose(ptr[:], vt[:, j * D:(j + 1) * D], ident[:])
                kt = sc * n_st + j
                # scale by r[token] during PSUM->SBUF copy (token = partition)
                nc.scalar.activation(v_sb[:, kt * D:(kt + 1) * D],
                                     ptr[:].bitcast(f32), AF.Copy,
                                     scale=r_pp[:, kt:kt + 1])

        # ---------------- pass 2: Q proj + attention + AG + proj ----------
        wpt_sb = []
        for t in range(HD // D):
            w = wpt_p.tile([D, OUTC], f32r, tag="wp")
            nc.sync.dma_start(w[:], wpt_d[t * D:(t + 1) * D, :].bitcast(f32r))
            wpt_sb.append(w)

        for sc in range(n_sc):
            ssl = slice(sc * SC, (sc + 1) * SC)
            pq = [psN.tile([D, SC], f32, tag="b", name=f"pq{_h}") for _h in range(NHQ)]
            for ht in range(n_ht):
                xt = x_p.tile([D, SC], f32r, tag="x")
                nc.sync.dma_start(xt[:], xT_d[ht * D:(ht + 1) * D, ssl].bitcast(f32r))
                st, sp = ht == 0, ht == n_ht - 1
                for h in range(NHQ):
                    nc.tensor.matmul(pq[h][:], wkvq_sb[ht][:, (2 + h) * D:(3 + h) * D],
                                     xt[:], start=st, stop=sp)
            cqt = trig_p.tile([D, SC], f32, tag="ck")
            sqt = trig_p.tile([D, SC], f32, tag="sk")
            nc.sync.dma_start(cqt[:], cq_d[:, ssl])
            nc.sync.dma_start(sqt[:], sq_d[:, ssl])
            qhat = []
            for h in range(NHQ):
                rb = rms_scale(pq[h], "q")
                qh = qh_p.tile([D, SC], f32r, tag="qh")
                rope_block(pq[h], cqt, sqt, qh[:], rb[:])
                qhat.append(qh)

            plan = chunk_plan[sc]
            for h in range(NHQ):
                pattn = psN.tile([D, SC], f32, tag="b")
                pden = psN.tile([1, SC], f32, tag="b")
                for i, (kt, mid) in enumerate(plan):
                    ps = psN.tile([D, SC], f32, tag="b")
                    nc.tensor.matmul(ps[:], khatT[:, kt * D:(kt + 1) * D],
                                     qhat[h][:], start=True, stop=True)
                    if mid is not None:
                        mt = mask_p.tile([D, SC], f32, tag="m")
                        nc.sync.dma_start(mt[:], masks_d[mid, :, :])
                        nc.vector.tensor_add(ps[:], ps[:], mt[:])
                    pe = pexp_p.tile([D, SC], f32r, tag="pe")
                    nc.scalar.activation(pe[:], ps[:], AF.Exp, scale=QKSCALE)
                    first, last = i == 0, i == len(plan) - 1
                    nc.tensor.matmul(pattn[:], v_sb[:, kt * D:(kt + 1) * D],
                                     pe[:], start=first, stop=last)
                    nc.tensor.matmul(pden[:], ones[:], pe[:], start=first,
                                     stop=last)

                at = at_p.tile([D, SC], f32, tag="at")
                if not plan:
                    nc.vector.memset(at[:], 0.0)
                else:
                    if uncov[sc] is not None:
                        nc.vector.tensor_add(pden[:], pden[:], uncov_sb[0:1, ssl])
                    dln = row_p.tile([1, SC], f32, tag="row")
                    nc.scalar.activation(dln[:], pden[:], AF.Ln)
                    rec = row_p.tile([1, SC], f32r, tag="rowr")
                    nc.scalar.activation(rec[:], dln[:], AF.Exp, scale=-1.0)
                    prb = psN.tile([D, SC], f32, tag="b")
                    nc.tensor.matmul(prb[:], onesr[:], rec[:], start=True, stop=True)
                    rb2 = rb_p.tile([D, SC], f32, tag="rb")
                    nc.scalar.copy(rb2[:], prb[:])
                    nc.vector.tensor_mul(at[:], pattn[:], rb2[:])
                nc.sync.dma_start(ag_in[sc][h * D:(h + 1) * D, :], at[:])

            nc.gpsimd.collective_compute(
                "AllGather", mybir.AluOpType.bypass,
                replica_groups=[list(range(NCORES))],
                ins=[ag_in[sc].ap()], outs=[ag_out[sc].ap()],
            )

        # ---------------- tail: output projection (transposed) ------------
        n_ct = OUTC // D
        for sc in range(n_sc):
            ssl = slice(sc * SC, (sc + 1) * SC)
            po = [psN.tile([D, SC], f32, tag="b", name=f"po{_j}") for _j in range(n_ct)]
            for t in range(HD // D):
                lt = lt_p.tile([D, SC], f32r, tag="lt")
                nc.sync.dma_start(lt[:], ag_out[sc][t * D:(t + 1) * D, :].bitcast(f32r))
                for j in range(n_ct):
                    nc.tensor.matmul(po[j][:], wpt_sb[t][:, j * D:(j + 1) * D],
                                     lt[:], start=(t == 0), stop=(t == HD // D - 1))
            for j in range(n_ct):
                ob = os_p.tile([D, SC], f32, tag="os")
                nc.scalar.copy(ob[:], po[j][:])
                nc.sync.dma_start(out_d[j * D:(j + 1) * D, ssl], ob[:])

    return nc


def build_and_run(x, cos, sin, pre_norm_w, q_norm_w, k_norm_w, Wq, Wk, Wv,
                  Wproj, q_ranges, k_ranges, cfg=None, trace=False,
                  trace_kwargs=None):
    from concourse.bass_utils import run_bass_kernel_spmd

    cfg = cfg or FULL_CFG
    per_core, spec = _host_prep(x, cos, sin, pre_norm_w, q_norm_w, k_norm_w,
                                Wq, Wk, Wv, Wproj, q_ranges, k_ranges, cfg)
    n_masks = per_core[0]["masks"].shape[0]
    nc = _build_program(cfg, spec, n_masks)
    _patch_bass(nc)

    in_maps = []
    for c in range(NCORES):
        m = dict(per_core[c])
        if any(u is not None for u in spec["uncov"]):
            S = cfg["S"]
            ua = np.zeros((1, S), np.float32)
            for sc, u in enumerate(spec["uncov"]):
                if u is not None:
                    ua[0, sc * SC:(sc + 1) * SC] = u
            m["uncov"] = ua
        in_maps.append(m)

    kw = {}
    if trace:
        kw = dict(trace=True, trace_kwargs=trace_kwargs or {})
    res = run_bass_kernel_spmd(nc, in_maps, core_ids=list(range(NCORES)), **kw)
    out = np.concatenate([res.results[c]["out"] for c in range(NCORES)], axis=0).T
    if not spec["all_covered"]:
        out = out * spec["covered"].T  # zero uncovered rows
    return out, res


def kernel(**inputs):
    out, _ = build_and_run(**inputs)
    return out
